# revision 11
# baseline (speedup 1.0000x reference)
"""AnchorLoss Trainium2 kernel.

loss = sum_{b,i,j: mask[b,i,j]==1} (1 - exp(-|z_i - z_j|^2 / 10)),  z = embedding + abs_coords

Sharding: data-parallel over batch B=8, one batch per NeuronCore. Each core:
  - device-side prep: z = e + a, r = |z|^2, bf16 hi/lo splits (pseudo-fp32),
  - streams its [2048, 2048] int32 mask in 16 row-blocks of [128, 2048],
  - per 1024-col chunk: K=10 bf16 matmul -> PSUM = d2 (hi/lo expansion),
    ScalarE exp with scale=-0.1, VectorE fused (E - 1) * mask with
    per-partition accumulate,
  - returns [128, 32] partial sums; host sums and negates.

The host passes e/a stacked+transposed+folded as one [16, N/4] array
(layout only, zero flops): row d*8+g holds [e_d chunk g | a_d chunk g],
so prep ops run 16-partition-wide (~0.4us each) and the coordinate load
is a single small DMA.
"""
import numpy as np
import sys

for _p in ("/opt/trn_rl_repo", "/root/.axon_site/_ro/trn_rl_repo"):
    if _p not in sys.path:
        sys.path.append(_p)

N = 2048
B = 8

_CACHED = None


def _build(n=N):
    from concourse import bacc, mybir, tile

    f32 = mybir.dt.float32
    i32 = mybir.dt.int32
    bf16 = mybir.dt.bfloat16
    AF = mybir.ActivationFunctionType
    ALU = mybir.AluOpType

    nb = n // 128          # mask row blocks
    cw = min(n, 1024)      # pipeline chunk width
    nch = n // cw          # chunks per row block
    nj = cw // 512         # matmuls per chunk

    G = 8                  # prep fold factor
    w = n // G             # folded chunk width
    nc = bacc.Bacc()
    ea_in = nc.declare_dram_parameter("ea", [2 * G, 2 * w], f32, isOutput=False)
    m_in = nc.declare_dram_parameter("m", [n, n], i32, isOutput=False)
    out = nc.declare_dram_parameter("out", [128, nb * nch], f32, isOutput=True)

    with tile.TileContext(nc) as tc:
        with (
            tc.tile_pool(name="singles", bufs=1) as singles,
            tc.tile_pool(name="maskp", bufs=8) as maskp,
            tc.tile_pool(name="ep", bufs=4) as ep,
            tc.tile_pool(name="psum", bufs=4, space="PSUM") as psump,
        ):
            # warm the ACT exp table set off the critical path
            dummy = singles.tile([1, 8], f32)
            nc.gpsimd.memset(dummy[:], 0.0)
            nc.scalar.activation(dummy[:], dummy[:], AF.Exp)

            # ---- prep (folded [2G, w] layout; row d*G+g = coord d, chunk g) ----
            ea = singles.tile([2 * G, 2 * w], f32)  # [e_d chunk g | a_d chunk g]
            nc.sync.dma_start(ea[:], ea_in[:])
            zt = singles.tile([2 * G, w], f32)
            nc.vector.tensor_tensor(zt[:], ea[:, 0:w], ea[:, w:2 * w], ALU.add)
            sq = singles.tile([2 * G, w], f32)
            nc.vector.tensor_tensor(sq[:], zt[:], zt[:], ALU.mult)

            # ---- bf16 hi/lo splits: hi = bf16(v), lo = bf16(v - hi) ----
            zh = singles.tile([2 * G, w], bf16)
            zl = singles.tile([2 * G, w], bf16)
            sqh = singles.tile([2 * G, w], bf16)
            sql = singles.tile([2 * G, w], bf16)
            m2zh = singles.tile([2 * G, w], bf16)   # -2 * zh (exact in bf16)
            m2zl = singles.tile([2 * G, w], bf16)
            nc.scalar.activation(zh[:], zt[:], AF.Copy)
            nc.vector.tensor_tensor(zl[:], zt[:], zh[:], ALU.subtract)
            nc.scalar.activation(sqh[:], sq[:], AF.Copy)
            nc.vector.tensor_tensor(sql[:], sq[:], sqh[:], ALU.subtract)
            nc.vector.tensor_scalar_mul(m2zh[:], zh[:], -2.0)
            nc.vector.tensor_scalar_mul(m2zl[:], zl[:], -2.0)
            ones4 = singles.tile([4, n], bf16)
            nc.vector.memset(ones4[:], 1.0)

            # K=14 row pairing (lhsT row k x rhs row k) -> PSUM = d2
            # (r_i + r_j enter as their four z^2 hi/lo components):
            #  k0-3:  1_i * [sqxh, sqyh, sqxl, sqyl]_j
            #  k4-7:  [sqxh, sqyh, sqxl, sqyl]_i * 1_j
            #  k8:  zxh*m2zxh  k9:  zyh*m2zyh  k10: zxh*m2zxl
            #  k11: zyh*m2zyl  k12: zxl*m2zxh  k13: zyl*m2zyh
            # Placement DMAs linearize folded [2G, w] (partition-major)
            # into [2, n] rows (free-major) -- same element order.
            zcol = singles.tile([14, n], bf16)  # rhs (j side)
            nc.scalar.dma_start(zcol[0:2, :], sqh[:])
            nc.scalar.dma_start(zcol[2:4, :], sql[:])
            nc.scalar.dma_start(zcol[4:8, :], ones4[:])
            nc.scalar.dma_start(zcol[8:10, :], m2zh[:])
            nc.scalar.dma_start(zcol[10:12, :], m2zl[:])
            nc.scalar.dma_start(zcol[12:14, :], m2zh[:])

            zrow = singles.tile([14, n], bf16)  # lhsT (i side)
            nc.scalar.dma_start(zrow[0:4, :], ones4[:])
            nc.scalar.dma_start(zrow[4:6, :], sqh[:])
            nc.scalar.dma_start(zrow[6:8, :], sql[:])
            nc.scalar.dma_start(zrow[8:10, :], zh[:])
            nc.scalar.dma_start(zrow[10:12, :], zh[:])
            nc.scalar.dma_start(zrow[12:14, :], zl[:])

            acc = singles.tile([128, nb * nch], f32)

            # ---- main loop: nb row blocks x nch chunks ----
            for ib in range(nb):
                mk = maskp.tile([128, n], bf16)
                nc.gpsimd.dma_start(mk[:], m_in[ib * 128:(ib + 1) * 128, :])
                for h in range(nch):
                    ps = psump.tile([128, cw], f32)
                    for jc in range(nj):
                        c0 = h * cw + jc * 512
                        nc.tensor.matmul(
                            ps[:, jc * 512:(jc + 1) * 512],
                            zrow[:, ib * 128:(ib + 1) * 128],
                            zcol[:, c0:c0 + 512],
                            start=True,
                            stop=True,
                        )
                    ev = ep.tile([128, cw], bf16)
                    nc.scalar.activation(ev[:], ps[:], AF.Exp, scale=-0.1)
                    nc.vector.scalar_tensor_tensor(
                        ev[:], ev[:], 1.0, mk[:, h * cw:(h + 1) * cw],
                        op0=ALU.subtract, op1=ALU.mult,
                        accum_out=acc[:, ib * nch + h:ib * nch + h + 1],
                    )
            nc.sync.dma_start(out[:], acc[:])
    nc.compile()
    return nc


def _get_graph():
    global _CACHED
    if _CACHED is None:
        _CACHED = _build()
    return _CACHED


def _pack_ea(e, a, n, G=8):
    w = n // G
    ea = np.empty((2 * G, 2 * w), dtype=np.float32)
    for d in range(2):
        ea[d * G:(d + 1) * G, :w] = e[:, d].reshape(G, w)
        ea[d * G:(d + 1) * G, w:] = a[:, d].reshape(G, w)
    return ea


def kernel(embedding, abs_coords, patch_mask, _trace=False, _trace_kwargs=None):
    from concourse.bass_utils import run_bass_kernel_spmd

    nc = _get_graph()
    in_maps = [
        {
            "ea": _pack_ea(embedding[b], abs_coords[b], N),
            "m": np.ascontiguousarray(patch_mask[b], dtype=np.int32),
        }
        for b in range(B)
    ]
    kw = {}
    if _trace:
        kw = dict(trace=True, **(_trace_kwargs or {}))
    res = run_bass_kernel_spmd(nc, in_maps, core_ids=list(range(B)), **kw)
    total = -sum(
        float(np.sum(r["out"], dtype=np.float64)) for r in res.results
    )
    out = np.float32(total)
    if _trace:
        return out, res
    return out
